# revision 8
# baseline (speedup 1.0000x reference)
"""KAN layer (nn_KANLayer) Trainium2 kernel, SPMD over 8 NeuronCores.

Math: out[o,n] = sum_i w_b[i,o]*silu(x[i,n])
              + sum_i w_s[i,o] * sum_c cp[i,o,c] * B_c(x[i,n])

The spline part M_{i,o}(x) = sum_c w_s*cp*B_c(x) is a C^2 piecewise cubic
on the uniform knot grid t_j (from grid_points).  On a window [t_J0, t_J1)
chosen at runtime to cover the actual x range exactly:

   M(x) = a0 + a1*x + a2*x^2 + a3*x^3 + sum_{j=J0+1}^{J1-1} g_j*relu(x-t_j)^3

so the whole layer collapses to F = 4 + n_knots dense feature planes + bias:

   out[o,n] = bias[o] + sum_{i,f} A[f,i,o] * Phi_f(x[i,n])
   Phi = [silu(x), x, x^2, x^3, relu(x-t_j)^3 ...]

A/bias are computed on host (float64) from w_b, w_s, grid_points,
control_points.  Device engine assignment per 1024-col core slice:
  ACT : silu, (x-t_j)^2 via Square(x + bias), PSUM->SBUF bias-add
  DVE : relu shifts (fused max/sub tensor_scalar, 2x mode), x^2, x^3,
        part of the cube multiplies
  Pool: remaining cube multiplies
  PE  : F matmul planes (contraction over i) into PSUM per 512-col group;
        silu/x planes in fp32, polynomial/cube planes in float32r
        (1 cyc/row) — producers write float32r directly (BIR rule).
Sharding: data-parallel over N (8192/8 = 1024 per core), A replicated.
"""

import numpy as np

import concourse.bacc as bacc
import concourse.tile as tile
import concourse.mybir as mybir
from concourse import bass_utils

AFT = mybir.ActivationFunctionType
ALU = mybir.AluOpType
F32 = mybir.dt.float32
F32R = mybir.dt.float32r

IN_DIM, OUT_DIM, N = 128, 128, 8192
N_CORES = 8
NS = N // N_CORES  # 1024 columns per core
HALF = 512         # PSUM group width

N_F32_PLANES = 2   # planes 0 (silu), 1 (x) run as plain fp32 matmuls
MM_F32R = True     # polynomial/cube planes as float32r (1 cyc/row)
POOL_CUBES = 3     # cube-multiplies on GPSIMD instead of DVE


def _build_planes(x, w_b, w_s, grid_points, control_points):
    """Host-side (float64) collapse of the spline to feature planes.

    Window [t_J0, t_J1) picked from the actual x range, so the truncated
    power representation is exact for every sample (no clipping needed).

    Returns A [F,128,128] f32 (A[f,i,o]), bias [128] f32, knots tuple.
    """
    t = np.asarray(grid_points, np.float64)
    xmin, xmax = float(np.min(x)), float(np.max(x))
    J0 = int(np.searchsorted(t, xmin, side="right") - 1)
    J1 = int(np.searchsorted(t, xmax, side="right"))  # xmax < t[J1]
    J0 = max(J0, 0)
    J1 = min(max(J1, J0 + 1), len(t) - 1)

    W = (np.asarray(w_s, np.float64)[:, :, None]
         * np.asarray(control_points, np.float64))  # (i,o,c)

    def coxdeboor(xv):
        xe = xv[..., None]
        B = ((xe >= t[:-1]) & (xe < t[1:])).astype(np.float64)
        for deg in range(1, 4):
            left = (xe - t[:-(deg + 1)]) / (t[deg:-1] - t[:-(deg + 1)])
            right = (t[deg + 1:] - xe) / (t[deg + 1:] - t[1:-deg])
            B = left * B[..., :-1] + right * B[..., 1:]
        return B

    coef = {}
    for j in range(J0, J1):
        xs = t[j] + (t[j + 1] - t[j]) * (
            0.5 + 0.5 * np.cos(np.pi * (np.arange(4) + 0.5) / 4))
        V = np.vander(xs, 4, increasing=True)
        coef[j] = np.linalg.solve(V, coxdeboor(xs))  # [4 powers, 65]

    a = np.einsum("ioc,mc->mio", W, coef[J0])  # base cubic on interval J0
    planes = [np.asarray(w_b, np.float64), a[1], a[2], a[3]]
    for j in range(J0 + 1, J1):
        planes.append(np.einsum("ioc,c->io", W, coef[j][3] - coef[j - 1][3]))
    A = np.stack(planes).astype(np.float32)      # [F,i,o]
    bias = a[0].sum(axis=0).astype(np.float32)   # [o]
    knots = tuple(float(v) for v in t[J0 + 1:J1])
    return A, bias, knots


def _emit_kernel(tc, o_d, x_d, a_d, b_d, knots):
    nc = tc.nc
    nk = len(knots)
    F = 4 + nk
    rdt = F32R if MM_F32R else F32
    with tc.tile_pool(name="sb", bufs=1) as pool, \
         tc.tile_pool(name="ps", bufs=1, space="PSUM") as psum:
        xs = pool.tile([128, NS], F32, name="xs")
        nc.sync.dma_start(xs, x_d)
        # planes 0..N_F32_PLANES-1 consumed as f32, the rest as f32r
        at2 = pool.tile([128, N_F32_PLANES * 128], F32, name="at2")
        nc.sync.dma_start(at2, a_d[:, :N_F32_PLANES * 128].bitcast(F32))
        at = pool.tile([128, (F - N_F32_PLANES) * 128], rdt, name="at")
        nc.sync.dma_start(at, a_d[:, N_F32_PLANES * 128:])
        bt = pool.tile([128, 1 + nk], F32, name="bt")
        nc.sync.dma_start(bt, b_d)

        phi = [None] * F
        dt_of = [F32 if f < N_F32_PLANES else rdt for f in range(F)]
        phi[0] = pool.tile([128, NS], dt_of[0], name="phi0")
        nc.scalar.activation(phi[0], xs, AFT.Silu)
        phi[1] = xs  # x-plane: raw input (f32 matmul)
        x2f = pool.tile([128, NS], F32, name="x2f")
        nc.vector.tensor_tensor(x2f, xs, xs, op=ALU.mult)          # x^2 f32
        phi[2] = pool.tile([128, NS], dt_of[2], name="phi2")
        nc.vector.tensor_scalar(phi[2], x2f, 1.0, None, op0=ALU.mult)  # round
        phi[3] = pool.tile([128, NS], dt_of[3], name="phi3")
        nc.vector.tensor_tensor(phi[3], x2f, xs, op=ALU.mult)      # x^3
        for k, tj in enumerate(knots):
            r = pool.tile([128, NS], F32, name=f"r{k}")
            # relu(x - tj) = (x max tj) - tj — fused DVE tensor_scalar (2x)
            nc.vector.tensor_scalar(r, xs, float(tj), float(-tj),
                                    op0=ALU.max, op1=ALU.add)
            # (x - tj)^2 on ACT, independent of r: Square(x + (-tj))
            r2 = pool.tile([128, NS], F32, name=f"r2_{k}")
            nc.scalar.activation(r2, xs, AFT.Square, bias=bt[:, 1 + k:2 + k])
            # relu(x-tj)^3 = (x-tj)^2 * relu(x-tj)
            phi[4 + k] = pool.tile([128, NS], dt_of[4 + k], name=f"phi{4 + k}")
            eng = nc.gpsimd if k < POOL_CUBES else nc.vector
            eng.tensor_tensor(phi[4 + k], r2, r, op=ALU.mult)

        outs = pool.tile([128, NS], F32, name="outs")
        for h in range(NS // HALF):
            acc = psum.tile([128, HALF], F32, name=f"acc{h}")
            sl = slice(h * HALF, (h + 1) * HALF)
            for f in range(F):
                if f < N_F32_PLANES:
                    lhsT = at2[:, f * 128:(f + 1) * 128]
                else:
                    lhsT = at[:, (f - N_F32_PLANES) * 128:(f - N_F32_PLANES + 1) * 128]
                nc.tensor.matmul(acc, lhsT, phi[f][:, sl],
                                 start=(f == 0), stop=(f == F - 1))
            # PSUM -> SBUF with per-partition bias[o]
            nc.scalar.activation(outs[:, sl], acc, AFT.Identity, bias=bt[:, 0:1])
            nc.sync.dma_start(o_d[:, sl], outs[:, sl])


_CACHE = {}


def _get_program(knots):
    key = (knots, MM_F32R, POOL_CUBES, N_F32_PLANES)
    if key in _CACHE:
        return _CACHE[key]
    F = 4 + len(knots)
    rdt = F32R if MM_F32R else F32
    nc = bacc.Bacc("TRN2", target_bir_lowering=False, debug=False,
                   num_devices=N_CORES)
    x_d = nc.dram_tensor("x", [128, NS], F32, kind="ExternalInput").ap()
    a_d = nc.dram_tensor("a", [128, F * 128], rdt, kind="ExternalInput").ap()
    b_d = nc.dram_tensor("b", [128, 1 + len(knots)], F32,
                         kind="ExternalInput").ap()
    o_d = nc.dram_tensor("o", [128, NS], F32, kind="ExternalOutput").ap()
    with tile.TileContext(nc) as tc:
        _emit_kernel(tc, o_d, x_d, a_d, b_d, knots)
    nc.compile()
    _CACHE[key] = nc
    return nc


def _run(nc, x, A_dram, bias_col, trace=False):
    in_maps = []
    for c in range(N_CORES):
        in_maps.append({
            "x": np.ascontiguousarray(x[:, c * NS:(c + 1) * NS]),
            "a": A_dram,
            "b": bias_col,
        })
    res = bass_utils.run_bass_kernel_spmd(
        nc, in_maps, core_ids=list(range(N_CORES)), trace=trace)
    out = np.concatenate([res.results[c]["o"] for c in range(N_CORES)], axis=1)
    return out, res


def _prep(x, w_b, w_s, grid_points, control_points):
    x = np.asarray(x, np.float32)
    A, bias, knots = _build_planes(x, w_b, w_s, grid_points, control_points)
    F = 4 + len(knots)
    A_dram = np.ascontiguousarray(A.transpose(1, 0, 2).reshape(128, F * 128))
    # column 0: output bias[o]; columns 1..nk: broadcast -t_j Square biases
    consts = np.concatenate(
        [bias[:, None]] +
        [np.full((128, 1), -tj, np.float32) for tj in knots], axis=1)
    bias_col = np.ascontiguousarray(consts.astype(np.float32))
    return x, A_dram, bias_col, knots


def kernel(x, w_b, w_s, grid_points, control_points):
    x, A_dram, bias_col, knots = _prep(x, w_b, w_s, grid_points, control_points)
    nc = _get_program(knots)
    out, _ = _run(nc, x, A_dram, bias_col)
    return out.astype(np.float32)
